# revision 25
# baseline (speedup 1.0000x reference)
"""Trainium2 Bass kernel for nn_BlockDirectTwice (dual-branch transformer block).

Sharding: data-parallel over batch. 8 batch elements -> 8 NeuronCores; every
core runs the full block (two LN+MHA branches, blend, LN, MLP, residuals) on
its own [S, D] slab. No collectives.

v2: fp8(e4m3) attention with DoubleRow matmuls (QKV/wo projections k-paired,
ctx t-paired), branch-interleaved emission so branch-1 attention overlaps
branch-0's softmax stream, LN apply offloaded to the scalar engine,
denominators DMA'd straight from PSUM. MLP stays bf16 (precision budget);
LayerNorm statistics, softmax denominators and the residual stream stay fp32.

The softmax is unnormalized exp (scores are small, no max-subtraction);
the per-query denominator rides the ctx matmul as a 65th "ones" row valued
1/32 so the reciprocal pre-scales ctx into fp8-friendly range; the wo-output
blend scale divides the 32 back out.
"""

import numpy as np
import ml_dtypes

B, S, D, H, DFF = 8, 1024, 768, 12, 3072
HD = D // H          # 64
P = 128
KD = D // P          # 6  K-subtiles over D
KF = DFF // P        # 24 K-subtiles over DFF
NT = S // P          # 8  token tiles
NPAIR = H // 2       # 6  head pairs
VW = 72              # v_aug per-head row width (65 used, padded for DoubleRow)
UP, MID = 0.6, 0.4
EPS = 1e-6
N_CORES = 8
ATT_SCALE = 1.0 / np.sqrt(HD)  # 0.125, folded into the exp activation
CINV = 1.0 / 32.0    # ones-row value: denominator prescale (ctx ends up x32)

_CACHE = {}


def _split_cols(n):
    """Split n output columns into <=512 chunks."""
    out, c = [], 0
    while c < n:
        w = min(512, n - c)
        out.append((c, w))
        c += w
    return out


def _build_nc(cfg):
    """Build the per-core Bass program. cfg is a frozenset of feature flags."""
    from contextlib import ExitStack

    import concourse.bass as bass
    import concourse.tile as tile
    from concourse import bacc, mybir

    F32 = mybir.dt.float32
    BF16 = mybir.dt.bfloat16
    FP8 = mybir.dt.float8e4
    AF = mybir.ActivationFunctionType
    ALU = mybir.AluOpType
    DR = mybir.MatmulPerfMode.DoubleRow

    has = lambda f: f in cfg
    repeat = 1
    for f in cfg:
        if f.startswith("repeat="):
            repeat = int(f.split("=")[1])

    nc = bacc.Bacc("TRN2", target_bir_lowering=False, debug=False)

    # ---------------- DRAM I/O ----------------
    x_dram = [
        nc.dram_tensor("x0", (S, D), F32, kind="ExternalInput"),
        nc.dram_tensor("x1", (S, D), F32, kind="ExternalInput"),
    ]
    # weights arrive host-reordered to match SBUF layouts exactly, so every
    # DMA is contiguous (>=768B per-partition lines):
    #   wq/wk: [NPAIR, P, KD, P] (pair-major slices), wv/wo: [P, KD, D]
    w_dram = {}
    for br in (0, 1):
        for nm in ("wq", "wk"):
            w_dram[(br, nm)] = nc.dram_tensor(f"a{br}_{nm}", (NPAIR, P, KD, P),
                                              FP8, kind="ExternalInput")
        for nm in ("wv", "wo"):
            w_dram[(br, nm)] = nc.dram_tensor(f"a{br}_{nm}", (P, KD, D), FP8,
                                              kind="ExternalInput")
    fc1_dram = nc.dram_tensor("fc1_w", (P, KD, DFF), BF16, kind="ExternalInput")
    fc2_dram = nc.dram_tensor("fc2_w", (P, KF, D), BF16, kind="ExternalInput")

    # optional non-trivial params (most are zeros/ones in this problem)
    opt_dram = {}
    for name, shape in [
        ("ln0_g", (D,)), ("ln0_b", (D,)), ("ln1_g", (D,)), ("ln1_b", (D,)),
        ("lnf_g", (D,)), ("lnf_b", (D,)),
        ("fc1_b", (DFF,)), ("fc2_b", (D,)),
        ("a0_bq", (D,)), ("a0_bk", (D,)), ("a0_bv", (D,)), ("a0_bo", (D,)),
        ("a1_bq", (D,)), ("a1_bk", (D,)), ("a1_bv", (D,)), ("a1_bo", (D,)),
    ]:
        if has(name):
            opt_dram[name] = nc.dram_tensor(name, shape, F32, kind="ExternalInput")

    out_dram = nc.dram_tensor("out", (S, D), F32, kind="ExternalOutput")

    def bcast_rows(src_ap, nparts):
        """DRAM row [1, n] (or [n]) -> AP broadcast over nparts partitions."""
        ap = list(src_ap.ap)
        if len(src_ap.shape) == 1:
            ap = [[0, nparts]] + ap
        else:
            ap = [[0, nparts]] + ap[1:]
        return bass.AP(tensor=src_ap.tensor, offset=src_ap.offset, ap=ap)

    with ExitStack() as ctx:
        tc = ctx.enter_context(tile.TileContext(nc))

        sb = ctx.enter_context(tc.tile_pool(name="sb", bufs=1))
        wpool = ctx.enter_context(tc.tile_pool(name="w", bufs=2))
        qkw = ctx.enter_context(tc.tile_pool(name="qkw", bufs=2))
        lnp = ctx.enter_context(tc.tile_pool(name="ln", bufs=2))
        qkp = ctx.enter_context(tc.tile_pool(name="qk", bufs=2))
        xtp = ctx.enter_context(tc.tile_pool(name="xt", bufs=1))
        prp = ctx.enter_context(tc.tile_pool(name="probs", bufs=3))
        outp = ctx.enter_context(tc.tile_pool(name="out", bufs=2))
        psmm = ctx.enter_context(tc.tile_pool(name="psmm", bufs=4, space="PSUM"))
        pssc = ctx.enter_context(tc.tile_pool(name="pssc", bufs=2, space="PSUM"))
        dram = ctx.enter_context(tc.tile_pool(name="dram", bufs=1, space="DRAM"))

        loop_cm = tc.For_i(0, repeat, 1) if repeat > 1 else None
        if loop_cm is not None:
            ctx.enter_context(loop_cm)

        # persistent big tensors (h kept bf16: the residual stream tolerates it
        # and it buys back 12KB/partition of SBUF)
        h_tm = sb.tile([P, NT, D], BF16, tag="h_tm")
        xT = [sb.tile([P, KD, S], FP8, tag=f"xT{br}", name=f"xT{br}")
              for br in (0, 1)]
        ctx_all = [sb.tile([P, KD, S], FP8, tag=f"ctx{br}", name=f"ctx{br}")
                   for br in (0, 1)]
        v_aug = [sb.tile([P, NT, H, VW], FP8, tag=f"v{br}", name=f"v{br}")
                 for br in (0, 1)]
        fc1_sb = sb.tile([P, KD, DFF], BF16, tag="fc1")
        fc2_sb = sb.tile([P, KF, D], BF16, tag="fc2")
        nc.gpsimd.dma_start(fc1_sb, fc1_dram.ap())
        nc.gpsimd.dma_start(fc2_sb, fc2_dram.ap())
        for br in (0, 1):
            nc.vector.memset(v_aug[br][:, :, :, 64:65], CINV)

        # optional broadcast tiles for per-feature (free-dim) params
        bcast_sb = {}
        for name in ("ln0_g", "ln0_b", "ln1_g", "ln1_b", "lnf_g", "lnf_b",
                     "a0_bv", "a1_bv", "a0_bo", "a1_bo", "fc2_b"):
            if has(name):
                t = sb.tile([P, D], F32, tag=f"bc_{name}")
                nc.gpsimd.dma_start(t, bcast_rows(opt_dram[name].ap(), P))
                bcast_sb[name] = t
        for name in ("a0_bo", "a1_bo"):
            if name in bcast_sb:  # wo output is x32 until the blend rescale
                nc.vector.tensor_scalar_mul(bcast_sb[name], bcast_sb[name], 32.0)
        # per-partition bias tiles (feature-major layouts)
        pp_sb = {}
        for name, kk in (("a0_bq", KD), ("a0_bk", KD), ("a1_bq", KD),
                         ("a1_bk", KD), ("fc1_b", KF)):
            if has(name):
                t = sb.tile([P, kk], F32, tag=f"pp_{name}")
                nc.sync.dma_start(t, opt_dram[name].ap().rearrange("(m p) -> p m", p=P))
                pp_sb[name] = t

        # identity for PE-mode transposes
        from concourse.masks import make_identity
        ident = sb.tile([P, P], BF16, tag="ident")
        make_identity(nc, ident)

        def emit_ln(x_f32, dest, dest_dt, t, gname, bname, newton=1):
            """LayerNorm x_f32 [P, D] -> dest[:, j, t*128:(t+1)*128] (transposed
            via PE). x_f32 is only read (not clobbered) on the trivial-g/b path."""
            stats = lnp.tile([P, 3, 6], F32, tag="stats")
            for sg in range(3):
                nc.vector.bn_stats(stats[:, sg, :], x_f32[:, sg * 256:(sg + 1) * 256])
            mv = lnp.tile([P, 2], F32, tag="mv")
            nc.vector.bn_aggr(mv, stats)
            # rstd = 1/sqrt(var+eps), DVE-only (quake init + Newton steps):
            # keeps the ACT table set untouched (exp stays resident).
            vh = lnp.tile([P, 1], F32, tag="rs_vh")
            nc.vector.tensor_scalar(vh, mv[:, 1:2], EPS, 0.5, ALU.add, ALU.mult)
            yi = lnp.tile([P, 1], mybir.dt.int32, tag="rs_yi")
            nc.vector.tensor_scalar(yi, mv[:, 1:2].bitcast(mybir.dt.int32), 1, None,
                                    ALU.logical_shift_right)
            y0 = lnp.tile([P, 1], F32, tag="rs_y0")
            nc.vector.tensor_scalar(yi, yi, -1, None, ALU.bitwise_xor)
            nc.vector.tensor_scalar(y0.bitcast(mybir.dt.int32), yi, 0x5f3759e0, None,
                                    ALU.add)
            t1 = lnp.tile([P, 1], F32, tag="rs_t1")
            for _ in range(newton):
                nc.vector.tensor_tensor(t1, y0, y0, ALU.mult)
                nc.vector.tensor_tensor(t1, t1, vh, ALU.mult)
                nc.vector.tensor_scalar(t1, t1, -1.0, 1.5, ALU.mult, ALU.add)
                nc.vector.tensor_tensor(y0, y0, t1, ALU.mult)
            xln = lnp.tile([P, D], BF16, tag="xln")
            if has(gname) or has(bname):
                xf = lnp.tile([P, D], F32, tag="xf")
                nc.vector.tensor_scalar(xf, x_f32, mv[:, 0:1], y0,
                                        ALU.subtract, ALU.mult)
                if has(gname):
                    nc.vector.tensor_tensor(xf, xf, bcast_sb[gname], ALU.mult)
                if has(bname):
                    nc.vector.tensor_tensor(xln, xf, bcast_sb[bname], ALU.add)
                else:
                    nc.vector.tensor_copy(xln, xf)
            else:
                # fused apply: xln = x*rstd + (-mu*rstd), one DVE op
                nmr = lnp.tile([P, 1], F32, tag="nmr")
                nc.vector.scalar_tensor_tensor(nmr, mv[:, 0:1], -1.0, y0,
                                               ALU.mult, ALU.mult)
                nc.vector.tensor_scalar(xln, x_f32, y0, nmr, ALU.mult, ALU.add)
            # transpose all KD chunks into one PSUM tile, single copy out
            pst = psmm.tile([P, 512], F32, tag="mm")
            pb = pst.bitcast(BF16)  # [P, 1024]
            for j in range(KD):
                nc.tensor.transpose(pb[:, j * P:(j + 1) * P],
                                    xln[:, j * P:(j + 1) * P], ident)
            src = pb[:, 0:KD * P].rearrange("p (j c) -> p j c", c=P)
            nc.vector.tensor_copy(dest[:, 0:KD, t * P:(t + 1) * P], src)

        def load_w(br, nm):
            t = wpool.tile([P, KD, D], FP8, tag="w768")
            nc.scalar.dma_start(t, w_dram[(br, nm)].ap())
            return t

        def dr_mm(ps, lhsT3, rhs3, start, stop):
            """One DoubleRow matmul over a k-pair ([P,2,m] APs), or two plain
            matmuls under the 'nodr' ablation flag."""
            if has("nodr"):
                nc.tensor.matmul(ps, lhsT=lhsT3[:, 0], rhs=rhs3[:, 0],
                                 start=start, stop=False)
                nc.tensor.matmul(ps, lhsT=lhsT3[:, 1], rhs=rhs3[:, 1],
                                 start=False, stop=stop)
            else:
                nc.tensor.matmul(ps, lhsT=lhsT3, rhs=rhs3, start=start,
                                 stop=stop, perf_mode=DR)

        def v_chunk(br, wv, t):
            """V projection for token tile t (DoubleRow, fp8)."""
            bias_key = f"a{br}_bv"
            for c0, cw in _split_cols(D):
                ps = psmm.tile([P, 512], F32, tag="mm")
                for k in range(0, KD, 2):
                    dr_mm(ps[:, :cw], xT[br][:, k:k + 2, t * P:(t + 1) * P],
                          wv[:, k:k + 2, c0:c0 + cw],
                          start=(k == 0), stop=(k == KD - 2))
                nh = cw // HD
                h0 = c0 // HD
                src = ps[:, :cw].rearrange("p (h c) -> p h c", c=HD)
                dst = v_aug[br][:, t, h0:h0 + nh, 0:HD]
                if bias_key in bcast_sb:
                    bcv = bcast_sb[bias_key][:, c0:c0 + cw].rearrange(
                        "p (h c) -> p h c", c=HD)
                    nc.vector.tensor_tensor(dst, src, bcv, ALU.add)
                else:
                    nc.vector.tensor_copy(dst, src)

        def stage_A(br):
            """Load x_br, blend into h_tm, LN, transpose into xT[br]; the
            V projection of tile t rides right behind tile t's transpose so
            the PE queue never blocks on the next tile's LN chain."""
            g, b = (f"ln{br}_g", f"ln{br}_b")
            wv = load_w(br, "wv")
            for t in range(NT):
                xt = lnp.tile([P, D], F32, tag="x_tm")
                nc.sync.dma_start(xt, x_dram[br].ap()[t * P:(t + 1) * P, :])
                if br == 0:
                    nc.vector.tensor_scalar_mul(h_tm[:, t, :], xt, UP)
                else:
                    nc.vector.scalar_tensor_tensor(h_tm[:, t, :], xt, MID,
                                                   h_tm[:, t, :], ALU.mult, ALU.add)
                emit_ln(xt, xT[br], FP8, t, g, b)
                v_chunk(br, wv, t)

        # Q/K projections, feature-major: out[feature_pair_rows, tokens], fp8.
        # Weights are loaded as per-pair [P, KD, 128] slices so only the live
        # pair's slices occupy SBUF (both branches interleave).
        def qk_chunks(br, pr):
            """4 closures emitting pair pr's q/k projections (DoubleRow)."""
            wslice = {}
            for which in ("q", "k"):
                wt = qkw.tile([P, KD, P], FP8, tag=f"{which}w{br}",
                              name=f"{which}w{br}_{pr}")
                nc.scalar.dma_start(wt, w_dram[(br, f"w{which}")].ap()[pr])
                wslice[which] = wt
            qp = qkp.tile([P, S], FP8, tag=f"qp{br}", name=f"qp{br}_{pr}")
            kp = qkp.tile([P, S], FP8, tag=f"kp{br}", name=f"kp{br}_{pr}")
            qk_tiles[(br, pr)] = (qp, kp)
            chunks = []
            for which, dest in (("q", qp), ("k", kp)):
                wt = wslice[which]
                for c0, cw in _split_cols(S):
                    def emit(which=which, wt=wt, dest=dest, c0=c0, cw=cw, pr=pr):
                        ps = psmm.tile([P, 512], F32, tag="mm")
                        for k in range(0, KD, 2):
                            dr_mm(ps[:, :cw], wt[:, k:k + 2, :],
                                  xT[br][:, k:k + 2, c0:c0 + cw],
                                  start=(k == 0), stop=(k == KD - 2))
                        nc.vector.tensor_copy(dest[:, c0:c0 + cw], ps[:, :cw])
                        bias_key = f"a{br}_b{which}"
                        if bias_key in pp_sb and c0 + cw >= S:
                            nc.vector.tensor_scalar_add(
                                dest, dest, pp_sb[bias_key][:, pr:pr + 1])
                    chunks.append(emit)
            return chunks

        qk_tiles = {}
        denom_dram = {br: dram.tile([H, S], F32, tag=f"denom{br}",
                                    name=f"denom{br}")
                      for br in (0, 1)}
        shared_pq = None
        if has("noexp"):
            shared_pq = sb.tile([P, 2, 2, 512], FP8, tag="pqshared")
            nc.vector.memset(shared_pq, 0.03)

        def attn_pair(br, pr, fillers, fill_start=0):
            """Scores/exp/ctx for (br, pr), interleaving filler closures so the
            in-order PE queue stays dense while ACT computes exps. fill_start
            delays filler pacing to step index >= fill_start (of 2*NT)."""
            qp, kp = qk_tiles[(br, pr)]
            nfill = 0
            for n in range(2):
                n0 = n * 512
                ps_c = [psmm.tile([P, 512], F32, tag="mm", name=f"ps_c{hh}")
                        for hh in range(2)]

                def ctx_pair(tp, pq):
                    for hh in range(2):
                        h = 2 * pr + hh
                        dr_mm(ps_c[hh][0:65, :],
                              v_aug[br][:, 2 * tp:2 * tp + 2, h, 0:65],
                              pq[:, 0:2, hh, :],
                              start=(tp == 0), stop=(tp == NT // 2 - 1))

                pending = []
                pq = None
                for t in range(NT):
                    ps_s = pssc.tile([P, 2, 512], F32, tag="sc")
                    for hh in range(2):
                        b0 = hh * HD
                        nc.tensor.matmul(
                            ps_s[:, hh, :],
                            lhsT=kp[b0:b0 + HD, t * P:(t + 1) * P],
                            rhs=qp[b0:b0 + HD, n0:n0 + 512],
                            start=True, stop=True)
                    if has("noexp"):
                        pq = shared_pq
                    else:
                        if t % 2 == 0:
                            pq = prp.tile([P, 2, 2, 512], FP8, tag="probs")
                        nc.scalar.activation(pq[:, t % 2], ps_s, AF.Exp,
                                             scale=float(ATT_SCALE))
                    if t % 2 == 1:
                        pending.append((t // 2, pq))
                        if len(pending) > 1:
                            ctx_pair(*pending.pop(0))
                    step = n * NT + t + 1
                    if step > fill_start:
                        want = ((step - fill_start) * len(fillers)) // (
                            2 * NT - fill_start)
                        while nfill < want:
                            fillers[nfill]()
                            nfill += 1
                for item in pending:
                    ctx_pair(*item)
                for hh in range(2):
                    h = 2 * pr + hh
                    nc.vector.tensor_copy(
                        ctx_all[br][hh * HD:(hh + 1) * HD, pr, n0:n0 + 512],
                        ps_c[hh][0:HD, :])
                    dstage = lnp.tile([65, 512], BF16, tag="dstage", bufs=1)
                    nc.vector.tensor_copy(dstage[64:65, :], ps_c[hh][64:65, :])
                    nc.gpsimd.dma_start(denom_dram[br][h:h + 1, n0:n0 + 512],
                                        dstage[64:65, :])
            while nfill < len(fillers):
                fillers[nfill]()
                nfill += 1

        def recip_scale(br):
            """denominators -> reciprocal -> broadcast multiply onto ctx."""
            recip_sb = sb.tile([H, S], F32, tag="recip")
            nc.gpsimd.dma_start(recip_sb, denom_dram[br][:])
            nc.vector.reciprocal_approx_fast(recip_sb, recip_sb)
            recip_dram = dram.tile([H, S], F32, tag="recipd")
            nc.sync.dma_start(recip_dram, recip_sb)
            for pr in range(NPAIR):
                rb = outp.tile([P, S], F32, tag="recipB", bufs=1)
                for hh in range(2):
                    h = 2 * pr + hh
                    nc.gpsimd.dma_start(rb[hh * HD:(hh + 1) * HD, :],
                                        bcast_rows(recip_dram[h:h + 1, :], HD))
                nc.vector.tensor_tensor(ctx_all[br][:, pr, :], ctx_all[br][:, pr, :],
                                        rb, ALU.mult)

        def wo_chunks(br, wo, per_t_tail=None):
            """Per-token-tile closures: wo projection + blend-add into h_tm."""
            scale = (UP if br == 0 else MID) * CINV
            bo_key = f"a{br}_bo"

            def emit_t(t):
                for c0, cw in _split_cols(D):
                    ps = psmm.tile([P, 512], F32, tag="mm")
                    for k in range(0, KD, 2):
                        dr_mm(ps[:, :cw], ctx_all[br][:, k:k + 2, t * P:(t + 1) * P],
                              wo[:, k:k + 2, c0:c0 + cw],
                              start=(k == 0), stop=(k == KD - 2))
                    if bo_key in bcast_sb:
                        tmp = lnp.tile([P, D], F32, tag="wo_tmp")
                        nc.vector.tensor_tensor(tmp[:, :cw], ps[:, :cw],
                                                bcast_sb[bo_key][:, c0:c0 + cw],
                                                ALU.add)
                        nc.vector.scalar_tensor_tensor(
                            h_tm[:, t, c0:c0 + cw], tmp[:, :cw], float(scale),
                            h_tm[:, t, c0:c0 + cw], ALU.mult, ALU.add)
                    else:
                        nc.vector.scalar_tensor_tensor(
                            h_tm[:, t, c0:c0 + cw], ps[:, :cw], float(scale),
                            h_tm[:, t, c0:c0 + cw], ALU.mult, ALU.add)
                if per_t_tail is not None:
                    per_t_tail(t)

            return [lambda t=t: emit_t(t) for t in range(NT)]

        # ---------------- emit program ----------------
        stage_A(0)
        for ch in qk_chunks(0, 0):
            ch()
        stage_A(1)
        for ch in qk_chunks(1, 0):
            ch()
        # prefetch wo weights now; the DMAs land during the attention phase
        wo_w = [load_w(0, "wo"), load_w(1, "wo")]
        wo0 = wo_chunks(0, wo_w[0])

        if not has("noattn"):
            for pr in range(NPAIR):
                attn_pair(0, pr, qk_chunks(0, pr + 1) if pr + 1 < NPAIR else [])
                if pr + 1 < NPAIR:
                    attn_pair(1, pr, qk_chunks(1, pr + 1))
                else:
                    # branch-0 ctx is scaled by now; feed part of wo0 into the
                    # last loop's second half (after recip0's DMA round trip)
                    recip_scale(0)
                    attn_pair(1, pr, wo0[:5], fill_start=NT)
            recip_scale(1)
            for ch in wo0[5:]:
                ch()  # covers recip1's DMA round trip
        else:
            for br in (0, 1):
                nc.vector.memset(ctx_all[br], 0.25)
            for ch in wo0:
                ch()

        # LNf -> hT (bf16 for the bf16 MLP), riding wo1's t-loop
        hT = xtp.tile([P, KD, S], BF16, tag="hT")
        for ch in wo_chunks(1, wo_w[1],
                            per_t_tail=lambda t: emit_ln(
                                h_tm[:, t, :], hT, BF16, t,
                                "lnf_g", "lnf_b", newton=2)):
            ch()

        # MLP: fc1+gelu then fc2+residual, in token chunks of 256
        if has("nomlp"):
            for t in range(NT):
                o_t = outp.tile([P, D], F32, tag="out_t", bufs=1)
                nc.vector.tensor_copy(o_t, h_tm[:, t, :])
                nc.gpsimd.dma_start(out_dram.ap()[t * P:(t + 1) * P, :], o_t)
        for nn in range(4 if not has("nomlp") else 0):
            c0 = nn * 256
            gT = xtp.tile([P, KF, 256], BF16, tag="gT")
            for m in range(KF):
                ps = psmm.tile([P, 512], F32, tag="mm")
                for k in range(KD):
                    nc.tensor.matmul(ps[:, :256], lhsT=fc1_sb[:, k, m * P:(m + 1) * P],
                                     rhs=hT[:, k, c0:c0 + 256],
                                     start=(k == 0), stop=(k == KD - 1))
                bias = pp_sb["fc1_b"][:, m:m + 1] if "fc1_b" in pp_sb else 0.0
                nc.scalar.activation(gT[:, m, :], ps[:, :256], AF.Gelu, bias=bias)
            for tl in range(2):
                t = 2 * nn + tl
                o_t = outp.tile([P, D], F32, tag="out_t", bufs=1)
                for oc0, ocw in _split_cols(D):
                    ps = psmm.tile([P, 512], F32, tag="mm")
                    for k in range(KF):
                        nc.tensor.matmul(
                            ps[:, :ocw], lhsT=gT[:, k, tl * P:(tl + 1) * P],
                            rhs=fc2_sb[:, k, oc0:oc0 + ocw],
                            start=(k == 0), stop=(k == KF - 1))
                    if "fc2_b" in bcast_sb:
                        nc.vector.tensor_tensor(ps[:, :ocw], ps[:, :ocw],
                                                bcast_sb["fc2_b"][:, oc0:oc0 + ocw],
                                                ALU.add)
                    nc.vector.tensor_tensor(o_t[:, oc0:oc0 + ocw], ps[:, :ocw],
                                            h_tm[:, t, oc0:oc0 + ocw], ALU.add)
                nc.gpsimd.dma_start(out_dram.ap()[t * P:(t + 1) * P, :], o_t)

    nc.compile()
    return nc


def _prep_inputs(inputs):
    """Host-side prep: detect trivial params, cast weights to fp8/bf16."""
    bf16 = ml_dtypes.bfloat16
    fp8 = ml_dtypes.float8_e4m3
    cfg = set()
    arrs = {}
    for name in ("x0", "x1"):
        arrs[name] = np.ascontiguousarray(np.asarray(inputs[name], dtype=np.float32))
    NP_, P_, KD_ = H // 2, 128, D // 128
    for br in (0, 1):
        for nm in ("wq", "wk"):
            key = f"a{br}_{nm}"
            w = np.asarray(inputs[key], dtype=np.float32).astype(fp8)
            # [ (ko p), (pr n) ] -> [pr, p, ko, n]: pair-major slices
            w = w.reshape(KD_, P_, NP_, P_).transpose(2, 1, 0, 3)
            arrs[key] = np.ascontiguousarray(w)
        for nm in ("wv", "wo"):
            key = f"a{br}_{nm}"
            w = np.asarray(inputs[key], dtype=np.float32).astype(fp8)
            # [ (ko p), n ] -> [p, ko, n]
            w = w.reshape(KD_, P_, D).transpose(1, 0, 2)
            arrs[key] = np.ascontiguousarray(w)
    arrs["fc1_w"] = np.ascontiguousarray(
        np.asarray(inputs["fc1_w"], dtype=np.float32).astype(bf16)
        .reshape(KD_, P_, DFF).transpose(1, 0, 2))
    arrs["fc2_w"] = np.ascontiguousarray(
        np.asarray(inputs["fc2_w"], dtype=np.float32).astype(bf16)
        .reshape(DFF // P_, P_, D).transpose(1, 0, 2))
    for name, trivial in [
        ("ln0_g", 1.0), ("ln0_b", 0.0), ("ln1_g", 1.0), ("ln1_b", 0.0),
        ("lnf_g", 1.0), ("lnf_b", 0.0), ("fc1_b", 0.0), ("fc2_b", 0.0),
        ("a0_bq", 0.0), ("a0_bk", 0.0), ("a0_bv", 0.0), ("a0_bo", 0.0),
        ("a1_bq", 0.0), ("a1_bk", 0.0), ("a1_bv", 0.0), ("a1_bo", 0.0),
    ]:
        a = np.asarray(inputs[name], dtype=np.float32)
        if not np.all(a == trivial):
            cfg.add(name)
            arrs[name] = np.ascontiguousarray(a)
    return cfg, arrs


def kernel(**inputs):
    from concourse.bass_utils import run_bass_kernel_spmd

    cfg, arrs = _prep_inputs(inputs)
    key = frozenset(cfg)
    if key not in _CACHE:
        _CACHE[key] = _build_nc(key)
    nc = _CACHE[key]

    shared = {k: v for k, v in arrs.items() if k not in ("x0", "x1")}
    in_maps = []
    for b in range(N_CORES):
        m = dict(shared)
        m["x0"] = np.ascontiguousarray(arrs["x0"][b])
        m["x1"] = np.ascontiguousarray(arrs["x1"][b])
        in_maps.append(m)

    res = run_bass_kernel_spmd(nc, in_maps, core_ids=list(range(N_CORES)))
    out = np.stack([res.results[b]["out"] for b in range(N_CORES)], axis=0)
    return out.astype(np.float32)
